# revision 17
# baseline (speedup 1.0000x reference)
"""Trainium2 Bass kernel for nn_AggregPolicy (GNN message passing / GRU chain).

Strategy:
  - Pure data parallelism: 524288 rows split across 8 cores (65536 each).
  - Feature-major on-chip layout: state s = [hj0..hj6, hm] (32 features) on
    partitions, batch on the free dim. 4 batch subgroups stacked on partitions
    (partition 32q+s) so elementwise ops use all 128 lanes.
  - Each GRU message-passing step's full linear algebra is a 32->128 linear map
    (neighbor structure folded into a banded weight matrix). Executed as 16
    small matmuls (K=32, M=32) with tile_position packing, writing gate-type-
    major PSUM banks: R | Z | INN | HN, each [128, 512].
  - Nonlinearities: ACT sigmoid/tanh with fused per-partition bias; DVE/GPSIMD
    for the remaining pointwise ops.
  - Iteration 1 consumes x directly (initial Linear layers folded into the
    first step's weights); final Linear folded into 4 output matmuls; final
    bias + layout restore on host.
"""

import sys
import numpy as np

for _p in ("/opt/trn_rl_repo",):
    if _p not in sys.path:
        sys.path.append(_p)

import ml_dtypes
from contextlib import ExitStack

import concourse.bass as bass
import concourse.bacc as bacc
import concourse.tile as tile
from concourse import mybir
from concourse.bass_utils import run_bass_kernel_spmd

BF16 = ml_dtypes.bfloat16
AF = mybir.ActivationFunctionType
ALU = mybir.AluOpType

N_CORES = 8
B = 524288
BC = B // N_CORES          # rows per core = 65536
NSUB = 4                   # batch subgroups stacked on partitions
NCOL = BC // NSUB          # free-dim columns per subgroup = 16384
CT = 512                   # columns per supertile (one PSUM bank)
NT = NCOL // CT            # 32 supertiles
H = 4
NU = 8                     # 7 joints + master
S = 32                     # state features


def _gate_blocks(p):
    """Build the 32->128 banded linear map for one message-passing step.

    Returns W (gate-major blocks) [4][32, 32] mapping state->gates and the
    four per-partition bias vectors (within one 32-wide subgroup block).
    Gate blocks: 0=R(sum), 1=Z(sum), 2=INN (input side of n), 3=HN (hidden
    side of n, bias excluded -- applied via STT scalar).
    State layout: [hj0(4) .. hj6(4), hm(4)].
    """
    Wih_j, Whh_j = p["Wih_j"], p["Whh_j"]
    Wih_m, Whh_m = p["Wih_m"], p["Whh_m"]
    W = [np.zeros((S, S), np.float64) for _ in range(4)]

    def st(u):  # state slice of unit u
        return slice(4 * u, 4 * u + 4)

    for u in range(7):
        left = None if u == 0 else st(u - 1)   # u==0 -> hm
        right = None if u == 6 else st(u + 1)  # u==6 -> zero
        for g, rows in ((0, slice(0, 4)), (1, slice(4, 8))):
            # sum gates: Wih(left,right) + Whh(self)
            Wl = Wih_j[rows, 0:4]
            Wr = Wih_j[rows, 4:8]
            Wh = Whh_j[rows, :]
            tgt = W[g][st(u), :]
            if left is None:
                tgt[:, 28:32] += Wl
            else:
                tgt[:, left] += Wl
            if right is not None:
                tgt[:, right] += Wr
            tgt[:, st(u)] += Wh
        # INN: input side only
        rows = slice(8, 12)
        tgt = W[2][st(u), :]
        if u == 0:
            tgt[:, 28:32] += Wih_j[rows, 0:4]
        else:
            tgt[:, st(u - 1)] += Wih_j[rows, 0:4]
        if u != 6:
            tgt[:, st(u + 1)] += Wih_j[rows, 4:8]
        # HN: hidden side only
        W[3][st(u), st(u)] += Whh_j[rows, :]

    # master unit (index 7, state rows 28:32); input = hj0, hidden = hm
    for g, rows in ((0, slice(0, 4)), (1, slice(4, 8))):
        W[g][28:32, 0:4] += Wih_m[rows, :]
        W[g][28:32, 28:32] += Whh_m[rows, :]
    W[2][28:32, 0:4] += Wih_m[8:12, :]
    W[3][28:32, 28:32] += Whh_m[8:12, :]

    def unit_bias(vec_j, vec_m, rows):
        b = np.zeros(S, np.float64)
        for u in range(7):
            b[st(u)] = vec_j[rows]
        b[28:32] = vec_m[rows]
        return b

    br = unit_bias(p["bih_j"], p["bih_m"], slice(0, 4)) + unit_bias(
        p["bhh_j"], p["bhh_m"], slice(0, 4))
    bz = unit_bias(p["bih_j"], p["bih_m"], slice(4, 8)) + unit_bias(
        p["bhh_j"], p["bhh_m"], slice(4, 8))
    binn = unit_bias(p["bih_j"], p["bih_m"], slice(8, 12))
    bhn = unit_bias(p["bhh_j"], p["bhh_m"], slice(8, 12))
    return W, (br, bz, binn, bhn)


def _a0_ext(p):
    """[32, 19] initial-linear map: state0 = A0e @ [x(18); 1]."""
    A = np.zeros((S, 19), np.float64)
    Wj, bj, Wm, bm = p["Wj"], p["bj"], p["Wm"], p["bm"]
    for u in range(7):
        A[4 * u:4 * u + 4, 4 + u] = Wj[:, 0]
        A[4 * u:4 * u + 4, 11 + u] = Wj[:, 1]
        A[4 * u:4 * u + 4, 18] = bj
    A[28:32, 0:4] = Wm
    A[28:32, 18] = bm
    return A


def _host_weights(inputs):
    p = {k: np.asarray(v, np.float64) for k, v in inputs.items() if k != "x"}
    W, (br, bz, binn, bhn) = _gate_blocks(p)
    A0e = _a0_ext(p)

    # wtb [128,128]: rows 32q+k (k<32) = state idx, cols 32g+m = gate out m of block g
    wtb = np.zeros((128, 128), np.float64)
    # wt1 [128,128]: iteration-1 gate weights consuming xe(19) directly
    wt1 = np.zeros((128, 128), np.float64)
    # a0t: diag blocks for S0 psum (iter-1 blend h operand)
    a0t = np.zeros((128, 128), np.float64)
    # wat: diag blocks for output linear (state -> 7 activations)
    wat = np.zeros((128, 128), np.float64)
    Wa = p["Wa"]  # [1, 4]
    for q in range(4):
        r0 = 32 * q
        for g in range(4):
            wtb[r0:r0 + 32, 32 * g:32 * g + 32] = W[g].T
            W1g = W[g] @ A0e  # [32, 19]
            wt1[r0:r0 + 19, 32 * g:32 * g + 32] = W1g.T
        a0t[r0:r0 + 19, r0:r0 + 32] = A0e.T
        for u in range(7):
            wat[r0 + 4 * u:r0 + 4 * u + 4, r0 + u] = Wa[0, :]

    def bias128(v):
        return np.tile(v, 4).astype(np.float32).reshape(128, 1)

    return {
        "wtb": wtb.astype(BF16), "wt1": wt1.astype(BF16),
        "a0t": a0t.astype(BF16), "wat": wat.astype(BF16),
        "br": bias128(br), "bz": bias128(bz),
        "binn": bias128(binn), "bhn": bias128(bhn),
    }, float(np.asarray(inputs["ba"]).reshape(-1)[0])


def _host_x(x):
    """x [B,18] fp32 -> per-core [128, NCOL] bf16 (partition 32q+k, k<19)."""
    xs = []
    for c in range(N_CORES):
        xc = np.asarray(x[c * BC:(c + 1) * BC], np.float32)
        arr = np.zeros((4, 32, NCOL), np.float32)
        arr[:, 0:18, :] = xc.reshape(4, NCOL, 18).transpose(0, 2, 1)
        arr[:, 18, :] = 1.0
        xs.append(arr.reshape(128, NCOL).astype(BF16))
    return xs


def _build_program(ncol=NCOL, nt=NT, n_iters=7):
    nc = bacc.Bacc("TRN2", target_bir_lowering=False, debug=False,
                   num_devices=N_CORES)
    f32 = mybir.dt.float32
    bf16 = mybir.dt.bfloat16

    xd = nc.dram_tensor("x_il", [128, ncol], bf16, kind="ExternalInput").ap()
    wtbd = nc.dram_tensor("wtb", [128, 128], bf16, kind="ExternalInput").ap()
    wt1d = nc.dram_tensor("wt1", [128, 128], bf16, kind="ExternalInput").ap()
    a0td = nc.dram_tensor("a0t", [128, 128], bf16, kind="ExternalInput").ap()
    watd = nc.dram_tensor("wat", [128, 128], bf16, kind="ExternalInput").ap()
    biasd = {k: nc.dram_tensor(k, [128, 1], f32, kind="ExternalInput").ap()
             for k in ("br", "bz", "binn", "bhn")}
    yd = nc.dram_tensor("y", [28, ncol], f32, kind="ExternalOutput").ap()

    with tile.TileContext(nc) as tc, ExitStack() as ctx:
        cpool = ctx.enter_context(tc.tile_pool(name="consts", bufs=1))
        spool = ctx.enter_context(tc.tile_pool(name="state", bufs=1))
        gpool = ctx.enter_context(tc.tile_pool(name="gates", bufs=4))
        ppool = ctx.enter_context(tc.tile_pool(name="pairs", bufs=3))
        opool = ctx.enter_context(tc.tile_pool(name="outsb", bufs=3))

        xt = spool.tile([128, ncol], bf16, tag="xt")
        nc.sync.dma_start(xt[:], xd[:])
        GRP = 4                      # supertiles per state/blend group
        ngrp = max(1, nt // GRP)
        # Per-group state tiles [128, GRP*CT]: fine deps + wide blend ops.
        sts = [spool.tile([128, GRP * CT], bf16, name=f"st{p}", tag=f"st{p}")
               for p in range(ngrp)]

        wtb = cpool.tile([128, 128], bf16, tag="wtb")
        nc.sync.dma_start(wtb[:], wtbd[:])
        wt1 = cpool.tile([128, 128], bf16, tag="wt1")
        nc.sync.dma_start(wt1[:], wt1d[:])
        a0t = cpool.tile([128, 128], bf16, tag="a0t")
        nc.sync.dma_start(a0t[:], a0td[:])
        wat = cpool.tile([128, 128], bf16, tag="wat")
        nc.sync.dma_start(wat[:], watd[:])
        bias = {}
        for k in ("br", "bz", "binn", "bhn"):
            bias[k] = cpool.tile([128, 1], f32, tag=k, name=f"b_{k}")
            nc.sync.dma_start(bias[k][:], biasd[k][:])

        # per-group z/n collection tiles so blend ops run at [128, GRP*CT]
        zp = {}
        npt = {}

        def front(it, t, psg, ps0):
            """MMs + sigmoids + STT + t2 + tanh for one supertile."""
            first = it == 0
            wt = wt1 if first else wtb
            kk = 19 if first else 32
            p, h = t // GRP, t % GRP
            G = psg.tile([128, 4 * CT], f32, tag="G", name=f"G_{it}_{t}")
            for g in range(4):
                for q in range(4):
                    r0 = 32 * q
                    rhs = (xt[r0:r0 + kk, t * CT:(t + 1) * CT] if first
                           else sts[p][r0:r0 + kk, h * CT:(h + 1) * CT])
                    nc.tensor.matmul(
                        G[r0:r0 + 32, g * CT:(g + 1) * CT],
                        wt[r0:r0 + kk, 32 * g:32 * g + 32],
                        rhs, start=True, stop=True,
                        tile_position=(r0, r0),
                    )
            S0 = None
            if first:
                S0 = ps0.tile([128, CT], f32, tag="S0", name=f"S0_{t}")
                for q in range(4):
                    r0 = 32 * q
                    nc.tensor.matmul(
                        S0[r0:r0 + 32, :],
                        a0t[r0:r0 + 19, r0:r0 + 32],
                        xt[r0:r0 + 19, t * CT:(t + 1) * CT],
                        start=True, stop=True,
                        tile_position=(r0, r0),
                    )
            r = gpool.tile([128, CT], bf16, tag="r", name=f"r_{it}_{t}")
            nc.scalar.activation(r[:], G[:, 0:CT], AF.Sigmoid,
                                 bias=bias["br"][:])
            if h == 0:
                zp[p] = ppool.tile([128, GRP * CT], bf16, tag="zp",
                                   name=f"zp_{it}_{p}")
                npt[p] = ppool.tile([128, GRP * CT], bf16, tag="npt",
                                    name=f"np_{it}_{p}")
            nc.scalar.activation(zp[p][:, h * CT:(h + 1) * CT],
                                 G[:, CT:2 * CT], AF.Sigmoid,
                                 bias=bias["bz"][:])
            t1 = gpool.tile([128, CT], bf16, tag="t1", name=f"t1_{it}_{t}")
            nc.vector.scalar_tensor_tensor(
                t1[:], G[:, 3 * CT:4 * CT], bias["bhn"][:], r[:],
                ALU.add, ALU.mult)
            t2 = gpool.tile([128, CT], bf16, tag="t2", name=f"t2_{it}_{t}")
            nc.vector.tensor_add(t2[:], t1[:], G[:, 2 * CT:3 * CT])
            return S0, t2, npt[p]

        def tanh_op(t2, npt_t, t):
            h = t % GRP
            nc.scalar.activation(npt_t[:, h * CT:(h + 1) * CT], t2[:],
                                 AF.Tanh, bias=bias["binn"][:])

        def blend0(t, S0, zt, nt_):
            """iter-0 per-supertile blend: h from S0 psum."""
            p, h = t // GRP, t % GRP
            cs = slice(h * CT, (h + 1) * CT)
            d = gpool.tile([128, CT], bf16, tag="d", name=f"d0_{t}")
            nc.vector.tensor_sub(d[:], S0[:], nt_[:, cs])
            e = gpool.tile([128, CT], bf16, tag="e", name=f"e0_{t}")
            nc.vector.tensor_mul(e[:], zt[:, cs], d[:])
            nc.vector.tensor_add(sts[p][:, cs], nt_[:, cs], e[:])

        def blend(it, p):
            """d, e, h' for one group, wide DVE ops [128, GRP*CT]."""
            z, n = zp_s[(it, p)], npt_s[(it, p)]
            d = ppool.tile([128, GRP * CT], bf16, tag="dp", name=f"d_{it}_{p}")
            nc.vector.tensor_sub(d[:], sts[p][:], n[:])
            e = ppool.tile([128, GRP * CT], bf16, tag="ep", name=f"e_{it}_{p}")
            nc.vector.tensor_mul(e[:], z[:], d[:])
            nc.vector.tensor_add(sts[p][:], n[:], e[:])

        zp_s = {}
        npt_s = {}

        # ---- iteration 0: per-tile skew-2 pipeline, G single-buffered
        with tc.tile_pool(name="ps0", bufs=3, space="PSUM") as ps0, \
             tc.tile_pool(name="psg1", bufs=1, space="PSUM") as psg1:
            pend = {}
            tpend = {}
            for t in range(nt):
                p = t // GRP
                S0, t2, npt_t = front(0, t, psg1, ps0)
                pend[t] = (S0, zp[p], npt[p])
                tpend[t] = (t2, npt_t)
                if t >= 1:
                    tanh_op(*tpend.pop(t - 1), t - 1)
                if t >= 2:
                    blend0(t - 2, *pend.pop(t - 2))
            tanh_op(*tpend.pop(nt - 1), nt - 1)
            for t in (nt - 2, nt - 1):
                blend0(t, *pend.pop(t))

        # ---- iterations 1-6: group rounds, skew-2, G double-buffered
        # blend lag must stay < ngrp or a later iteration's matmuls would be
        # emitted (and hence ordered) before this iteration's state write
        lag = min(2, ngrp - 1) if ngrp >= 2 else 0
        with tc.tile_pool(name="psg", bufs=2, space="PSUM") as psg:
            rounds = [(it, p) for it in range(1, n_iters) for p in range(ngrp)]
            nb = 0
            tq = []
            for R, (it, p) in enumerate(rounds):
                for h in range(GRP):
                    _, t2, npt_t = front(it, GRP * p + h, psg, None)
                    tq.append((t2, npt_t, GRP * p + h))
                    if len(tq) > 1:
                        tanh_op(*tq.pop(0))
                zp_s[(it, p)], npt_s[(it, p)] = zp[p], npt[p]
                while nb <= R - lag:
                    blend(*rounds[nb])
                    nb += 1
            while tq:
                tanh_op(*tq.pop(0))
            while nb < len(rounds):
                blend(*rounds[nb])
                nb += 1

        # ---- output linear
        with tc.tile_pool(name="pso", bufs=2, space="PSUM") as pso:
            for t in range(nt):
                p, h = t // GRP, t % GRP
                O = pso.tile([128, CT], f32, tag="O", name=f"O_{t}")
                for q in range(4):
                    r0 = 32 * q
                    nc.tensor.matmul(
                        O[r0:r0 + 32, :],
                        wat[r0:r0 + 28, r0:r0 + 32],
                        sts[p][r0:r0 + 28, h * CT:(h + 1) * CT],
                        start=True, stop=True,
                        tile_position=(r0, r0),
                    )
                osb = opool.tile([128, CT], f32, tag="osb", name=f"osb_{t}")
                nc.scalar.copy(osb[:], O[:])
                for q in range(4):
                    nc.sync.dma_start(yd[7 * q:7 * q + 7, t * CT:(t + 1) * CT],
                                      osb[32 * q:32 * q + 7, :])

    nc.compile()
    return nc


_NC_CACHE = {}


def kernel(**inputs):
    x = np.asarray(inputs["x"])
    wd, ba = _host_weights(inputs)
    xs = _host_x(x)

    if "prog" not in _NC_CACHE:
        _NC_CACHE["prog"] = _build_program()
    nc = _NC_CACHE["prog"]

    in_maps = []
    for c in range(N_CORES):
        m = {"x_il": xs[c]}
        m.update({k: wd[k] for k in ("wtb", "wt1", "a0t", "wat",
                                     "br", "bz", "binn", "bhn")})
        in_maps.append(m)

    res = run_bass_kernel_spmd(nc, in_maps, core_ids=list(range(N_CORES)))
    _NC_CACHE["last_result"] = res
    outs = []
    for c in range(N_CORES):
        yc = np.asarray(res.results[c]["y"], np.float32)  # [28, NCOL]
        oc = yc.reshape(4, 7, NCOL).transpose(0, 2, 1).reshape(BC, 7)
        outs.append(oc)
    out = np.concatenate(outs, 0).reshape(B, 7, 1) + np.float32(ba)
    return out.astype(np.float32)


if __name__ == "__main__":
    rng = np.random.default_rng(0)
    demo = {"x": rng.standard_normal((B, 18), dtype=np.float32)}
    for k, shp in [("Wj", (H, 2)), ("bj", (H,)), ("Wm", (H, H)), ("bm", (H,)),
                   ("Wih_j", (3 * H, 2 * H)), ("Whh_j", (3 * H, H)),
                   ("bih_j", (3 * H,)), ("bhh_j", (3 * H,)),
                   ("Wih_m", (3 * H, H)), ("Whh_m", (3 * H, H)),
                   ("bih_m", (3 * H,)), ("bhh_m", (3 * H,)),
                   ("Wa", (1, H)), ("ba", (1,))]:
        demo[k] = (rng.standard_normal(shp) * 0.1).astype(np.float32)
    y = kernel(**demo)
    print(y.shape, y.dtype)
